# revision 5
# baseline (speedup 1.0000x reference)
"""BindingPocketGNN (3-layer GCN, N=50000, E=800000) on 8 Trainium2 NeuronCores.

Distribution: nodes sharded into 8 contiguous ranges (6250/core). Each core owns the
scatter/aggregation for its destination-node range; edges are routed (host-side) to the
core owning their destination. Source features come from a replicated node-major table
(host pre-gathered edge-ordered stream for layer 1; AllGather-replicated activation
tables for layers 2/3, fetched per-edge with large batched dma_gather ops).

Per layer, on each core (feat-major formulation so BN/bias are per-partition):
    msg[e, f_in]  = table[src_e]                  (layer 1: sequential stream;
                                                   layers 2/3: SWDGE dma_gather, int16
                                                   idxs against lo/hi table halves)
    z^T[f_in, d]  = sum_e msg_e[f_in] * mask[e,d] (TensorE: msg.T @ (iota==dloc) mask)
    zs            = z^T * dinv[dst]               (DVE mult with broadcast dinv rows)
    y^T[f_out, d] = W.T @ zs                      (TensorE)
    stats         = AllReduce(sum/sumsq of y)     (1KB collective; BN layers)
    act^T         = Relu(A*y^T + B)               (ScalarE, per-partition A/B)
    h             = act^T transposed to node-major, * dinv  (TensorE transpose + DVE)
    table_{l+1}   = AllGather(h)                  (collective; layers 1,2)
Layer 3 feeds a [128->1] FC matmul; +fcb and sigmoid applied on host.

Edge layout: per (core, dst-tile, lo/hi src half) segments padded to 128-edge chunks.
Chunk order: per group of GS dst tiles: all lo chunks (tile-major), then all hi chunks,
so each group needs exactly 2 dma_gather calls (table rows [0,32768) and [32768,N)).
dloc/idx/msg1 arrays share this chunk order; masks kill pad lanes (dloc=1000).
"""
import sys
if "/opt/trn_rl_repo" not in sys.path:
    sys.path.insert(0, "/opt/trn_rl_repo")

import os

import numpy as np
import ml_dtypes

import concourse.bass as bass
import concourse.bacc as bacc
import concourse.mybir as mybir
import concourse.tile as tile
from concourse import bass_utils

N = 50000
E = 800000
IN, HID = 64, 128
BN_EPS = 1e-5
NCORES = 8
NPC = N // NCORES          # 6250 nodes per core
P = 128
NT = (NPC + P - 1) // P    # 49 dst tiles per core
LAST_D = NPC - (NT - 1) * P  # 106
SPLIT = 32768              # int16 idx limit: table split into [0,SPLIT) and [SPLIT,N)
GS = 7                     # dst tiles per gather group
NG = (NT + GS - 1) // GS   # 7 groups
GCAP = int(os.environ.get("GCN_GCAP", "8"))  # chunks per dma_gather (ring capacity)

F32 = mybir.dt.float32
I32 = mybir.dt.int32
I16 = mybir.dt.int16
BF16 = mybir.dt.bfloat16
NP_BF16 = np.dtype(ml_dtypes.bfloat16)

REPS = int(os.environ.get("GCN_REPS", "1"))

Alu = mybir.AluOpType
Act = mybir.ActivationFunctionType

_NC_CACHE = {}


def _layout(S_lo, S_hi):
    """Static chunk layout. Returns (groups, base_lo, base_hi, TOTC).
    groups[g] = dict(base, C_lo, C_hi, C, tiles=[(t, [local chunk idxs])]).
    base_lo[t]/base_hi[t] = global chunk index of tile t's first lo/hi chunk."""
    base_lo = [0] * NT
    base_hi = [0] * NT
    groups = []
    gb = 0
    for g in range(NG):
        ts = list(range(g * GS, min((g + 1) * GS, NT)))
        off = 0
        for t in ts:
            base_lo[t] = gb + off
            off += S_lo[t]
        C_lo = off
        for t in ts:
            base_hi[t] = gb + off
            off += S_hi[t]
        C = off
        tiles = []
        for t in ts:
            lo_l = [base_lo[t] - gb + i for i in range(S_lo[t])]
            hi_l = [base_hi[t] - gb + i for i in range(S_hi[t])]
            tiles.append((t, lo_l + hi_l))
        groups.append(dict(base=gb, C_lo=C_lo, C_hi=C - C_lo, C=C, tiles=tiles))
        gb += C
    return groups, base_lo, base_hi, gb


def _build(S_lo, S_hi):
    """Build+schedule the SPMD program (identical for all 8 cores)."""
    groups, _, _, TOTC = _layout(S_lo, S_hi)
    nc = bacc.Bacc("TRN2", target_bir_lowering=False, debug=False, num_devices=NCORES)

    # ---- I/O ----
    msg1_d = nc.dram_tensor("msg1", [P, TOTC, IN], BF16, kind="ExternalInput")
    idx_d = nc.dram_tensor("idx16", [P, TOTC * 8], I16, kind="ExternalInput")
    dloc_d = nc.dram_tensor("dloc", [P, TOTC], F32, kind="ExternalInput")
    dinv_d = nc.dram_tensor("dinv_sl", [P, NT], F32, kind="ExternalInput")
    dinvb_d = nc.dram_tensor("dinv_bc", [P, NT * P], F32, kind="ExternalInput")
    iota_d = nc.dram_tensor("iota_bf", [P, P], BF16, kind="ExternalInput")
    ident_d = nc.dram_tensor("ident", [P, P], F32, kind="ExternalInput")
    W_d = [
        nc.dram_tensor("W1", [IN, HID], F32, kind="ExternalInput"),
        nc.dram_tensor("W2", [HID, HID], F32, kind="ExternalInput"),
        nc.dram_tensor("W3", [HID, HID], F32, kind="ExternalInput"),
    ]
    fcW_d = nc.dram_tensor("fcW", [HID, 1], F32, kind="ExternalInput")
    g_d = [nc.dram_tensor("g1", [HID, 1], F32, kind="ExternalInput"),
           nc.dram_tensor("g2", [HID, 1], F32, kind="ExternalInput")]
    bt_d = [nc.dram_tensor("bt1", [HID, 1], F32, kind="ExternalInput"),
            nc.dram_tensor("bt2", [HID, 1], F32, kind="ExternalInput")]
    b3_d = nc.dram_tensor("b3", [HID, 1], F32, kind="ExternalInput")
    outv = nc.dram_tensor("outv", [1, NPC], F32, kind="ExternalOutput")

    with tile.TileContext(nc) as tc:
        with (
            tc.tile_pool(name="meta", bufs=1) as meta,
            tc.tile_pool(name="m1p", bufs=2) as m1p,
            tc.tile_pool(name="m2p", bufs=2) as m2p,
            tc.tile_pool(name="maskp", bufs=12) as maskp,
            tc.tile_pool(name="zsp", bufs=3) as zsp,
            tc.tile_pool(name="actp", bufs=3) as actp,
            tc.tile_pool(name="hp", bufs=3) as hp,
            tc.tile_pool(name="sqp", bufs=2) as sqp,
            tc.tile_pool(name="fcsb_p", bufs=2) as fcsb_p,
            tc.tile_pool(name="zps_p", bufs=2, space="PSUM") as zps_p,
            tc.tile_pool(name="yps_p", bufs=2, space="PSUM") as yps_p,
            tc.tile_pool(name="trps_p", bufs=2, space="PSUM") as trps_p,
            tc.tile_pool(name="fcps_p", bufs=1, space="PSUM") as fcps_p,
            tc.tile_pool(name="dram", bufs=1, space="DRAM") as dram,
        ):
            # ---- resident metadata ----
            idx_sb = meta.tile([P, TOTC * 8], I16)
            nc.sync.dma_start(idx_sb[:], idx_d[:])
            dloc_sb = meta.tile([P, TOTC], F32)
            nc.sync.dma_start(dloc_sb[:], dloc_d[:])
            dinv_sl = meta.tile([P, NT], F32)
            nc.sync.dma_start(dinv_sl[:], dinv_d[:])
            dinv_bc = meta.tile([P, NT * P], F32)
            nc.sync.dma_start(dinv_bc[:], dinvb_d[:])
            iota_t = meta.tile([P, P], BF16)
            nc.sync.dma_start(iota_t[:], iota_d[:])
            ident = meta.tile([P, P], F32)
            nc.sync.dma_start(ident[:], ident_d[:])
            W_sb = []
            for l in range(3):
                fi = IN if l == 0 else HID
                w = meta.tile([fi, HID], F32, name=f"W{l}_sb")
                nc.sync.dma_start(w[:], W_d[l][:])
                W_sb.append(w)
            fcW_sb = meta.tile([HID, 1], F32)
            nc.sync.dma_start(fcW_sb[:], fcW_d[:])
            g_sb, bt_sb = [], []
            for l in range(2):
                gg = meta.tile([HID, 1], F32, name=f"g{l}_sb")
                nc.sync.dma_start(gg[:], g_d[l][:])
                g_sb.append(gg)
                bb = meta.tile([HID, 1], F32, name=f"bt{l}_sb")
                nc.sync.dma_start(bb[:], bt_d[l][:])
                bt_sb.append(bb)
            b3_sb = meta.tile([HID, 1], F32)
            nc.sync.dma_start(b3_sb[:], b3_d[:])
            eps_sb = meta.tile([P, 1], F32)
            nc.vector.memset(eps_sb[:], BN_EPS)

            ystore = meta.tile([P, NT * P], F32)
            sums = meta.tile([P, NT], F32)
            sumsq = meta.tile([P, NT], F32)

            # internal DRAM for collectives (fresh per rep: Shared allows one writer)
            def mk_coll(rep):
                tab_in = [dram.tile([NPC, HID], BF16, name=f"tab{l}_in_r{rep}") for l in (1, 2)]
                tab_out = [dram.tile([N, HID], BF16, name=f"tab{l}_out_r{rep}",
                                     addr_space="Shared") for l in (1, 2)]
                st_in = [dram.tile([P, 2], F32, name=f"st{l}_in_r{rep}") for l in (0, 1)]
                st_out = [dram.tile([P, 2], F32, name=f"st{l}_out_r{rep}", addr_space="Shared")
                          for l in (0, 1)]
                return tab_in, tab_out, st_in, st_out

            for _rep in range(REPS):
              tab_in, tab_out, st_in, st_out = mk_coll(_rep)
              for l in range(3):
                 f_in = IN if l == 0 else HID
                 # ---- aggregation + weight matmul, by gather group ----
                 for g, G in enumerate(groups):
                     a, C, C_lo, C_hi = G["base"], G["C"], G["C_lo"], G["C_hi"]
                     if l == 0:
                         mt = m1p.tile([P, C, IN], BF16, tag="m1")
                         nc.sync.dma_start(mt[:], msg1_d[:, a:a + C, :])
                         fdim = IN
                     else:
                         table = tab_out[l - 1]
                         mt = m2p.tile([P, C, HID], BF16, tag="m2")
                         for c0, c1, tv in (
                             (0, C_lo, table[0:SPLIT, :]),
                             (C_lo, C, table[SPLIT:N, :]),
                         ):
                             for cs in range(c0, c1, GCAP):
                                 ce = min(cs + GCAP, c1)
                                 nn = (ce - cs) * P
                                 nc.gpsimd.dma_gather(
                                     mt[:, cs:ce, :], tv,
                                     idx_sb[:, (a + cs) * 8:(a + ce) * 8],
                                     nn, nn, HID,
                                 )
                         fdim = HID
                     for t, chunks in G["tiles"]:
                         d_hi = LAST_D if t == NT - 1 else P
                         zps = zps_p.tile([P, P], F32, tag="zps")
                         nk = len(chunks)
                         for k, cl in enumerate(chunks):
                             mask = maskp.tile([P, P], BF16, tag="mask")
                             nc.vector.tensor_scalar(
                                 out=mask[:], in0=iota_t[:],
                                 scalar1=dloc_sb[:, a + cl:a + cl + 1],
                                 scalar2=None, op0=Alu.is_equal,
                             )
                             nc.tensor.matmul(zps[:f_in, :], lhsT=mt[:, cl, :], rhs=mask[:],
                                              start=(k == 0), stop=(k == nk - 1))
                         zs = zsp.tile([P, P], F32, tag="zs")
                         nc.vector.tensor_tensor(
                             out=zs[:f_in, :], in0=zps[:f_in, :],
                             in1=dinv_bc[:f_in, t * P:(t + 1) * P], op=Alu.mult,
                         )
                         yps = yps_p.tile([P, P], F32, tag="yps")
                         nc.tensor.matmul(yps[:], lhsT=W_sb[l][:], rhs=zs[:f_in, :],
                                          start=True, stop=True)
                         if l < 2:
                             nc.scalar.activation(
                                 out=ystore[:, t * P:t * P + d_hi], in_=yps[:, :d_hi],
                                 func=Act.Copy, accum_out=sums[:, t:t + 1],
                             )
                             sq = sqp.tile([P, P], F32, tag="sq")
                             nc.scalar.activation(
                                 out=sq[:, :d_hi], in_=yps[:, :d_hi],
                                 func=Act.Square, accum_out=sumsq[:, t:t + 1],
                             )
                         else:
                             act3 = actp.tile([P, P], F32, tag="act")
                             nc.scalar.activation(out=act3[:, :d_hi], in_=yps[:, :d_hi],
                                                  func=Act.Relu, bias=b3_sb[:], scale=1.0)
                             fcp = fcps_p.tile([1, P], F32, tag="fcp")
                             nc.tensor.matmul(fcp[:1, :d_hi], lhsT=fcW_sb[:], rhs=act3[:, :d_hi],
                                              start=True, stop=True)
                             fcs = fcsb_p.tile([1, P], F32, tag="fcs")
                             nc.vector.tensor_copy(fcs[:1, :d_hi], fcp[:1, :d_hi])
                             nc.sync.dma_start(outv[:1, t * P:t * P + d_hi], fcs[:1, :d_hi])

                 if l < 2:
                     # ---- BN stats allreduce + coefficients ----
                     stats = meta.tile([P, 2], F32, name=f"stats{l}_r{_rep}")
                     nc.vector.tensor_reduce(stats[:, 0:1], sums[:], axis=mybir.AxisListType.X, op=Alu.add)
                     nc.vector.tensor_reduce(stats[:, 1:2], sumsq[:], axis=mybir.AxisListType.X, op=Alu.add)
                     nc.sync.dma_start(st_in[l][:], stats[:])
                     nc.gpsimd.collective_compute(
                         "AllReduce", Alu.add, replica_groups=[list(range(NCORES))],
                         ins=[st_in[l][:]], outs=[st_out[l][:]],
                     )
                     tot = meta.tile([P, 2], F32, name=f"tot{l}_r{_rep}")
                     nc.sync.dma_start(tot[:], st_out[l][:])
                     cf = meta.tile([P, 6], F32, name=f"cf{l}_r{_rep}")  # mean ex2 var std A B
                     nc.vector.tensor_scalar_mul(cf[:, 0:1], tot[:, 0:1], 1.0 / N)
                     nc.vector.tensor_scalar_mul(cf[:, 1:2], tot[:, 1:2], 1.0 / N)
                     nc.vector.tensor_tensor(out=cf[:, 2:3], in0=cf[:, 0:1], in1=cf[:, 0:1], op=Alu.mult)
                     nc.vector.tensor_tensor(out=cf[:, 2:3], in0=cf[:, 1:2], in1=cf[:, 2:3], op=Alu.subtract)
                     nc.scalar.activation(out=cf[:, 3:4], in_=cf[:, 2:3], func=Act.Sqrt, bias=eps_sb[:], scale=1.0)
                     nc.vector.reciprocal(cf[:, 4:5], cf[:, 3:4])
                     A = meta.tile([P, 1], F32, name=f"A{l}_r{_rep}")
                     B = meta.tile([P, 1], F32, name=f"B{l}_r{_rep}")
                     nc.vector.tensor_tensor(out=A[:], in0=g_sb[l][:], in1=cf[:, 4:5], op=Alu.mult)
                     nc.vector.tensor_tensor(out=cf[:, 5:6], in0=cf[:, 0:1], in1=A[:], op=Alu.mult)
                     nc.vector.tensor_tensor(out=B[:], in0=bt_sb[l][:], in1=cf[:, 5:6], op=Alu.subtract)

                     # ---- epilogue: act, transpose to node-major, store table slice ----
                     for t in range(NT):
                         d_hi = LAST_D if t == NT - 1 else P
                         act = actp.tile([P, P], F32, tag="act")
                         nc.scalar.activation(out=act[:, :d_hi], in_=ystore[:, t * P:t * P + d_hi],
                                              func=Act.Relu, bias=B[:], scale=A[:])
                         tr = trps_p.tile([P, P], F32, tag="tr")
                         nc.tensor.transpose(tr[:d_hi, :], act[:, :d_hi], ident[:])
                         h = hp.tile([P, HID], BF16, tag="h")
                         nc.vector.tensor_scalar_mul(h[:d_hi, :], tr[:d_hi, :], dinv_sl[:d_hi, t:t + 1])
                         nc.sync.dma_start(tab_in[l][t * P:t * P + d_hi, :], h[:d_hi, :])
                     nc.gpsimd.collective_compute(
                         "AllGather", Alu.bypass, replica_groups=[list(range(NCORES))],
                         ins=[tab_in[l][:]], outs=[tab_out[l][:]],
                     )

    nc.compile()
    return nc


def _prep(inputs):
    x = np.asarray(inputs["x"], np.float32)
    ei = np.asarray(inputs["edge_index"], np.int64)
    loops = np.arange(N, dtype=np.int64)
    src = np.concatenate([ei[0], loops])
    dst = np.concatenate([ei[1], loops])
    deg = np.bincount(dst, minlength=N).astype(np.float32)
    dinv = (1.0 / np.sqrt(deg)).astype(np.float32)
    xs16 = (x * dinv[:, None]).astype(NP_BF16)

    core = dst // NPC
    rem = dst - core * NPC
    tidx = rem >> 7
    loc = (rem & 127).astype(np.float32)
    hi = (src >= SPLIT).astype(np.int64)

    order = np.lexsort((hi, tidx, core))
    src_s = src[order]
    core_s = core[order]
    tidx_s = tidx[order]
    hi_s = hi[order]
    loc_s = loc[order]

    seg = (core_s * NT + tidx_s) * 2 + hi_s
    cnt = np.bincount(seg, minlength=NCORES * NT * 2)
    S_lo = np.zeros(NT, np.int64)
    S_hi = np.zeros(NT, np.int64)
    cview = cnt.reshape(NCORES, NT, 2)
    ch = np.ceil(cview / P).astype(np.int64)
    S_lo = ch[:, :, 0].max(axis=0)
    S_hi = ch[:, :, 1].max(axis=0)
    for t in range(NT):
        if S_lo[t] + S_hi[t] == 0:
            S_lo[t] = 1
    S_lo_t = tuple(int(v) for v in S_lo)
    S_hi_t = tuple(int(v) for v in S_hi)

    groups, base_lo, base_hi, TOTC = _layout(S_lo_t, S_hi_t)
    base_lo = np.asarray(base_lo, np.int64)
    base_hi = np.asarray(base_hi, np.int64)

    starts = np.zeros(NCORES * NT * 2, np.int64)
    starts[1:] = np.cumsum(cnt)[:-1]
    pos = np.arange(len(src_s)) - starts[seg]
    cc = np.where(hi_s == 1, base_hi[tidx_s], base_lo[tidx_s]) + (pos >> 7)
    lane = pos & 127

    val = np.where(hi_s == 1, src_s - SPLIT, src_s).astype(np.int16)
    idxs_full = np.zeros((NCORES, TOTC, P), np.int16)
    idxs_full[core_s, cc, lane] = val
    dloc_full = np.full((NCORES, TOTC, P), 1000.0, np.float32)
    dloc_full[core_s, cc, lane] = loc_s
    msg1_full = np.zeros((NCORES, TOTC, P, IN), NP_BF16)
    msg1_full[core_s, cc, lane] = xs16[src_s]

    # device layouts
    dloc_dev = dloc_full.transpose(0, 2, 1).copy()            # [c, P, TOTC]
    msg1_dev = msg1_full.transpose(0, 2, 1, 3).copy()         # [c, P, TOTC, IN]
    # idx16: value of seq pos k=(cc*128+lane) at [16m + lane%16, cc*8 + lane//16]
    tmp = idxs_full.reshape(NCORES, TOTC, 8, 16)              # [c, cc, lane//16, lane%16]
    idx_dev = np.tile(tmp.transpose(0, 3, 1, 2).reshape(NCORES, 16, TOTC * 8),
                      (1, 8, 1)).copy()                       # [c, 128, TOTC*8]

    dv = dinv.reshape(NCORES, NPC)
    dinv_pad = np.zeros((NCORES, NT * P), np.float32)
    dinv_pad[:, :NPC] = dv
    dinv_sl = dinv_pad.reshape(NCORES, NT, P).transpose(0, 2, 1).copy()   # [c, P, NT]
    dinv_bc = np.broadcast_to(dinv_pad[:, None, :], (NCORES, P, NT * P)).copy()

    iota_bf = np.broadcast_to(np.arange(P, dtype=np.float32)[None, :], (P, P)).astype(NP_BF16)
    ident = np.eye(P, dtype=np.float32)

    com = {
        "W1": np.asarray(inputs["W1"], np.float32),
        "W2": np.asarray(inputs["W2"], np.float32),
        "W3": np.asarray(inputs["W3"], np.float32),
        "fcW": np.asarray(inputs["fcW"], np.float32).reshape(HID, 1),
        "g1": np.asarray(inputs["g1"], np.float32).reshape(HID, 1),
        "g2": np.asarray(inputs["g2"], np.float32).reshape(HID, 1),
        "bt1": np.asarray(inputs["bt1"], np.float32).reshape(HID, 1),
        "bt2": np.asarray(inputs["bt2"], np.float32).reshape(HID, 1),
        "b3": np.asarray(inputs["b3"], np.float32).reshape(HID, 1),
        "iota_bf": np.ascontiguousarray(iota_bf),
        "ident": ident,
    }
    in_maps = []
    for c in range(NCORES):
        m = dict(com)
        m["msg1"] = np.ascontiguousarray(msg1_dev[c])
        m["idx16"] = np.ascontiguousarray(idx_dev[c])
        m["dloc"] = np.ascontiguousarray(dloc_dev[c])
        m["dinv_sl"] = np.ascontiguousarray(dinv_sl[c])
        m["dinv_bc"] = np.ascontiguousarray(dinv_bc[c])
        in_maps.append(m)
    return in_maps, TOTC, (S_lo_t, S_hi_t)


def _get_nc(T, S):
    key = (T, S, REPS, GCAP)
    if key not in _NC_CACHE:
        _NC_CACHE[key] = _build(list(S[0]), list(S[1]))
    return _NC_CACHE[key]


class _Exec:
    """jit-once / device_put-once executor mirroring bass2jax.run_bass_via_pjrt."""

    def __init__(self, nc, in_maps):
        import jax
        from jax.sharding import Mesh, PartitionSpec
        from jax.experimental.shard_map import shard_map
        from concourse import bass2jax
        bass2jax.install_neuronx_cc_hook()
        n_cores = NCORES
        part_name = nc.partition_id_tensor.name if nc.partition_id_tensor else None
        in_names, out_names, out_avals, zero_outs = [], [], [], []
        for alloc in nc.m.functions[0].allocations:
            if not isinstance(alloc, mybir.MemoryLocationSet):
                continue
            name = alloc.memorylocations[0].name
            if alloc.kind == "ExternalInput":
                if name != part_name:
                    in_names.append(name)
            elif alloc.kind == "ExternalOutput":
                out_names.append(name)
                shape = tuple(alloc.tensor_shape)
                dtype = mybir.dt.np(alloc.dtype)
                out_avals.append(jax.core.ShapedArray(shape, dtype))
                zero_outs.append(np.zeros(shape, dtype))
        n_params = len(in_names)
        all_names = in_names + out_names
        if part_name is not None:
            all_names = all_names + [part_name]
        self.out_names, self.out_avals, self.n_cores = out_names, out_avals, n_cores

        def _body(*args):
            operands = list(args)
            if part_name is not None:
                operands.append(bass2jax.partition_id_tensor())
            outs = bass2jax._bass_exec_p.bind(
                *operands,
                out_avals=tuple(out_avals),
                in_names=tuple(all_names),
                out_names=tuple(out_names),
                lowering_input_output_aliases=(),
                sim_require_finite=True,
                sim_require_nnan=True,
                nc=nc,
            )
            return tuple(outs)

        devices = jax.devices()[:n_cores]
        mesh = Mesh(np.asarray(devices), ("core",))
        in_specs = (PartitionSpec("core"),) * (n_params + len(out_names))
        out_specs = (PartitionSpec("core"),) * len(out_names)
        self.fn = jax.jit(
            shard_map(_body, mesh=mesh, in_specs=in_specs, out_specs=out_specs,
                      check_rep=False),
            keep_unused=True,
        )
        concat_in = [
            np.concatenate([np.asarray(in_maps[c][k]) for c in range(n_cores)], axis=0)
            for k in in_names
        ]
        concat_zeros = [
            np.zeros((n_cores * z.shape[0], *z.shape[1:]), z.dtype) for z in zero_outs
        ]
        sh = jax.sharding.NamedSharding(mesh, PartitionSpec("core"))
        self.dev_in = [jax.device_put(a, sh) for a in concat_in] + \
                      [jax.device_put(a, sh) for a in concat_zeros]
        for a in self.dev_in:
            a.block_until_ready()

    def run(self):
        outs = self.fn(*self.dev_in)
        for o in outs:
            o.block_until_ready()
        return outs

    def results(self):
        outs = self.run()
        res = [dict() for _ in range(self.n_cores)]
        for i, name in enumerate(self.out_names):
            arr = np.asarray(outs[i]).reshape(self.n_cores, *self.out_avals[i].shape)
            for c in range(self.n_cores):
                res[c][name] = arr[c]
        return res


_EXEC_CACHE = {}


def _get_exec(in_maps, T, S):
    key = (T, S, REPS, GCAP)
    if key not in _EXEC_CACHE:
        _EXEC_CACHE[key] = _Exec(_get_nc(T, S), in_maps)
    return _EXEC_CACHE[key]


def _run(in_maps, T, S):
    nc = _get_nc(T, S)
    r = bass_utils.run_bass_kernel_spmd(nc, in_maps, core_ids=list(range(NCORES)), trace=False)
    return r


def kernel(**inputs):
    in_maps, T, S = _prep(inputs)
    r = _run(in_maps, T, S)
    out = np.concatenate([r.results[c]["outv"].reshape(-1) for c in range(NCORES)])
    fcb = np.asarray(inputs["fcb"], np.float32).reshape(-1)
    out = (out + fcb[0]).astype(np.float32)[:, None]
    # numerically stable sigmoid in fp32
    sig = np.empty_like(out)
    pos = out >= 0
    sig[pos] = 1.0 / (1.0 + np.exp(-out[pos], dtype=np.float32))
    ex = np.exp(out[~pos], dtype=np.float32)
    sig[~pos] = ex / (1.0 + ex)
    return out, sig
